# revision 1
# baseline (speedup 1.0000x reference)
"""Trainium2 Bass kernel for nn_Attention_83081847374268 (sparse sliding-window GQA).

Sharding: 8 cores = batch (2, data parallel) x kv-head (4, tensor parallel).
Each core computes, for its (b, kh): q/k/v projections (2 q heads, 1 kv head),
QK-RMSNorm + RoPE, banded sliding-window attention, and a partial output
projection against its 512-row slice of wout.  The host sums the 4 partials
per batch (the TP reduction) and stacks the batches.

Device dataflow (per core):
  stage A: stream xT column-chunks; matmul projections directly in transposed
           layout (qT/kT [head_dim, T]); RMSNorm via ones-matmul variance +
           PE-broadcast rstd; RoPE fused with the rstd multiply on DVE.
           v computed in natural layout [T, 256] and bounced via DRAM.
  stage B: per 128-query tile: S = qT.T @ kT over a host-chosen key window,
           additive mask bias (host-precomputed, handles any attn_mask /
           positions pattern), exp on ACT with fused row-sum, normalize on
           DVE, PE-transpose P, PV matmuls accumulate encoded^T.
  stage C: out partial = encT.T @ wout_slice, DMA per 128-row tile.

All matmuls run as float32r (TRN2 rounds fp32r operands to 12 mantissa bits
on write; full speed at moving-dim >= 256).
"""
import sys

sys.path.insert(0, "/opt/trn_rl_repo")

import numpy as np
import ml_dtypes

import concourse.bacc as bacc
import concourse.mybir as mybir
from concourse.bass_utils import run_bass_kernel_spmd
from concourse.tile import TileContext
from concourse.alu_op_type import AluOpType

F32 = mybir.dt.float32
F32R = mybir.dt.float32r
BF16 = mybir.dt.bfloat16
ACTF = mybir.ActivationFunctionType

B, T, WIDTH = 2, 2048, 2048
NUM_HEADS, NUM_KV_HEADS, HEAD_DIM = 8, 4, 256
GROUPS = NUM_HEADS // NUM_KV_HEADS  # 2 q heads per kv head (= per core)
WINDOW = 512
ROPE_BASE = 10000.0
ALPHA = HEAD_DIM ** -0.5
MASK_NEG = -100.0  # exp(S/16 + MASK_NEG) == 0 for |S|<=~16; exact in bf16

NT = T // 128           # 16 query tiles
TCH = 512               # stage-A t-chunk width
NTCH = T // TCH         # 4
NW = WIDTH // 128       # 16 contraction chunks

_prog_cache = {}
DEBUG_TAPS = False


def _round_up(x, m):
    return (x + m - 1) // m * m


def _geometry(positions, attn_mask):
    """Per-query-tile key windows from the actual mask/positions data."""
    pos = np.asarray(positions)
    am = np.asarray(attn_mask)
    pd = pos[:, :, None].astype(np.int64) - pos[:, None, :].astype(np.int64)
    valid = am & (np.abs(pd) < WINDOW)  # [B, T, T] bool
    assert valid.any(axis=2).all(), "a query row with no valid key is unsupported"
    js = []
    wmax = 0
    for it in range(NT):
        cols = valid[:, it * 128:(it + 1) * 128, :].any(axis=(0, 1))
        idx = np.nonzero(cols)[0]
        j_lo, j_hi = int(idx[0]), int(idx[-1]) + 1
        j0 = (j_lo // 128) * 128
        wmax = max(wmax, j_hi - j0)
        js.append(j0)
    Wb = max(256, _round_up(wmax, 128))
    Wb = min(Wb, T)
    js = tuple(max(0, min(j, T - Wb)) for j in js)
    return valid, Wb, js


def _pieces(Wb):
    """Split Wb into moving-dim pieces from {512, 384, 256} (fp32r full speed
    needs N >= 256; Wb is a multiple of 128 and >= 256)."""
    out = []
    rem = Wb
    while rem > 640:
        out.append(512)
        rem -= 512
    if rem == 128:
        out[-1] = 384
        rem = 256
    if rem > 512:
        out.append(384)
        rem -= 384
    if rem:
        out.append(rem)
    return out


def _rope_tables(pos_b, scale):
    """cos/sin tables in [head_dim/2, T] (transposed) layout, gain folded in."""
    d = np.arange(HEAD_DIM // 2, dtype=np.float32)
    timescale = (ROPE_BASE ** (2.0 / HEAD_DIM * d)).astype(np.float32)
    rad = pos_b.astype(np.float32)[None, :] / timescale[:, None]  # [128, T]
    cos, sin = np.cos(rad).astype(np.float32), np.sin(rad).astype(np.float32)
    g1 = (1.0 + scale[:HEAD_DIM // 2]).astype(np.float32)[:, None]
    g2 = (1.0 + scale[HEAD_DIM // 2:]).astype(np.float32)[:, None]
    # o1 = a1*C1 - a2*S2 ; o2 = a2*C2 + a1*S1
    return (cos * g1, sin * g1, cos * g2, sin * g2)  # C1, S1, C2, S2


def _build(Wb, js, shared_tables, debug_taps=False):
    nc = bacc.Bacc("TRN2", target_bir_lowering=False, debug=False, num_devices=8)

    def din(name, shape, dt):
        return nc.dram_tensor(name, shape, dt, kind="ExternalInput").ap()

    xT = din("xT", [WIDTH, T], F32R)
    wq = din("wq", [WIDTH, 512], F32R)
    wk = din("wk", [WIDTH, 256], F32R)
    wv = din("wv", [WIDTH, 256], F32R)
    wout = din("wout", [512, T], F32R)
    ident_d = din("ident", [128, 128], F32R)
    ones1_d = din("ones1", [1, 128], F32R)    # K=1 broadcast lhsT
    onesc_d = din("onesc", [128, 1], F32R)    # partition-sum lhsT
    bias_d = din("bias", [NT, 128, Wb], BF16)
    tab_names = ["ct", "st"] if shared_tables else [
        "cq1", "sq1", "cq2", "sq2", "ck1", "sk1", "ck2", "sk2"]
    tabs = {n: din(n, [128, T], F32) for n in tab_names}
    yp = nc.dram_tensor("yp", [T, T], F32, kind="ExternalOutput").ap()
    v_dram = nc.dram_tensor("v_scratch", [T, 256], F32R).ap()

    taps = {}
    if debug_taps:
        for nm, shape in (("qT_tap", [512, T]), ("kT_tap", [256, T]),
                          ("v_tap", [T, 256]), ("encT_tap", [512, T])):
            taps[nm] = nc.dram_tensor(nm, shape, F32, kind="ExternalOutput").ap()

    pieces = _pieces(Wb)
    NJ = Wb // 128  # P-transpose blocks per tile
    # per-group (2 query tiles) union of key chunks, and which halves exist
    groups = []
    for g in range(NT // 2):
        w0 = set(range(js[2 * g] // 128, js[2 * g] // 128 + NJ))
        w1 = set(range(js[2 * g + 1] // 128, js[2 * g + 1] // 128 + NJ))
        groups.append([(jc, jc in w0, jc in w1) for jc in sorted(w0 | w1)])

    with TileContext(nc) as tc:
        with (
            tc.tile_pool(name="persist", bufs=1) as pp,
            tc.tile_pool(name="qk_store", bufs=1) as qkp,
        ):
            ident = pp.tile([128, 128], F32R)
            nc.sync.dma_start(out=ident[:], in_=ident_d[:])
            ones1 = pp.tile([1, 128], F32R)
            nc.sync.dma_start(out=ones1[:], in_=ones1_d[:])
            onesc = pp.tile([128, 1], F32R)
            nc.sync.dma_start(out=onesc[:], in_=onesc_d[:])
            epsb = pp.tile([1, 1], F32)
            nc.any.memset(epsb[:], 1e-6)
            epsbq = pp.tile([1, 1], F32)
            nc.any.memset(epsbq[:], HEAD_DIM * 1e-6)
            ones_f = pp.tile([1, 1], F32)
            nc.any.memset(ones_f[:], 1.0)
            rstdq_c = [pp.tile([128, NT], F32, tag=f"rstdq{hh}", name=f"rstdq{hh}")
                       for hh in range(2)]
            zrow_f = pp.tile([128, 128], F32)
            nc.any.memset(zrow_f[:], 0.0)
            zero_r = pp.tile([128, 128], F32R)
            nc.vector.tensor_copy(zero_r[:], zrow_f[:])

            qT = [qkp.tile([128, T], F32R, tag=f"qT{c}", name=f"qT{c}") for c in range(4)]
            kT = [qkp.tile([128, T], F32R, tag=f"kT{c}", name=f"kT{c}") for c in range(2)]

            # ---------------- stage A: projections + RMSNorm + RoPE ----------
            ps_shared = tc.tile_pool(name="ps_shared", bufs=2, space="PSUM")
            psA = psA1 = psS = psT = psE = ps_shared.__enter__()
            with (
                tc.tile_pool(name="wpool", bufs=1) as wp,
                tc.tile_pool(name="xpool", bufs=2) as xp,
                tc.tile_pool(name="tabpool", bufs=1) as tp,
                tc.tile_pool(name="sa", bufs=1) as sa,
            ):
                wq_t = wp.tile([128, NW * 512], F32R)
                wk_t = wp.tile([128, NW * 256], F32R)
                wv_t = wp.tile([128, NW * 256], F32R)
                wq_r = wq.rearrange("(c p) m -> p c m", p=128)
                wk_r = wk.rearrange("(c p) m -> p c m", p=128)
                wv_r = wv.rearrange("(c p) m -> p c m", p=128)
                wq_v = wq_t[:].rearrange("p (c m) -> p c m", m=512)
                wk_v = wk_t[:].rearrange("p (c m) -> p c m", m=256)
                wv_v = wv_t[:].rearrange("p (c m) -> p c m", m=256)
                xT_r = xT.rearrange("(c p) t -> p c t", p=128)

                def load_xts(tci):
                    t0 = tci * TCH
                    xts = xp.tile([128, NW * TCH], F32R, tag="xts", name=f"xts{tci}")
                    xv = xts[:].rearrange("p (c t) -> p c t", t=TCH)
                    for q4 in range(4):
                        nc.sync.dma_start(
                            out=xv[:, q4 * 4:(q4 + 1) * 4],
                            in_=xT_r[:, q4 * 4:(q4 + 1) * 4, t0:t0 + TCH],
                        )
                    return xts

                xts_pre = xp.tile([128, NW * TCH], F32R, tag="xts", name="xts0")
                xv0 = xts_pre[:].rearrange("p (c t) -> p c t", t=TCH)
                for q4 in range(4):
                    nc.sync.dma_start(out=wk_v[:, q4 * 4:(q4 + 1) * 4],
                                      in_=wk_r[:, q4 * 4:(q4 + 1) * 4])
                    nc.sync.dma_start(out=xv0[:, q4 * 4:(q4 + 1) * 4],
                                      in_=xT_r[:, q4 * 4:(q4 + 1) * 4, 0:TCH])
                for q4 in range(4):
                    nc.sync.dma_start(out=wq_v[:, q4 * 4:(q4 + 1) * 4],
                                      in_=wq_r[:, q4 * 4:(q4 + 1) * 4])

                if shared_tables:
                    q_tabs = k_tabs = ("ct", "st", "ct", "st")
                else:
                    q_tabs = ("cq1", "sq1", "cq2", "sq2")
                    k_tabs = ("ck1", "sk1", "ck2", "sk2")
                units = [
                    (wk_t, 256, 0, k_tabs, kT, 0, None),
                    (wq_t, 512, 0, q_tabs, qT, 0, 0),
                    (wq_t, 512, 256, q_tabs, qT, 2, 1),
                ]
                for tci in range(NTCH):
                    t0 = tci * TCH
                    xts = xts_pre if tci == 0 else load_xts(tci)
                    # q/k projections in transposed layout + norm + rope
                    tabt = {}
                    for name in dict.fromkeys(q_tabs + k_tabs):
                        tt = tp.tile([128, TCH], F32, tag=name, name=f"tab_{name}")
                        nc.sync.dma_start(out=tt[:], in_=tabs[name][:, t0:t0 + TCH])
                        tabt[name] = tt
                    if tci == 0:
                        for q4 in range(4):
                            nc.sync.dma_start(out=wv_v[:, q4 * 4:(q4 + 1) * 4],
                                              in_=wv_r[:, q4 * 4:(q4 + 1) * 4])
                    for w_t, wcols, cbase, tkeys, dest, dbase, qhead in units:
                        ps1 = psA.tile([128, TCH], F32, tag="t_s0")
                        ps2 = psA.tile([128, TCH], F32, tag="t_s1")
                        for ps, cc in ((ps1, 0), (ps2, 1)):
                            coff = cbase + cc * 128
                            for wc in range(NW):
                                nc.tensor.matmul(
                                    ps[:],
                                    w_t[:, wc * wcols + coff: wc * wcols + coff + 128],
                                    xts[:, wc * TCH:(wc + 1) * TCH],
                                    start=(wc == 0), stop=(wc == NW - 1),
                                )
                        sq1 = sa.tile([128, TCH], F32R, tag="sq1")
                        sq2 = sa.tile([128, TCH], F32R, tag="sq2")
                        nc.scalar.activation(sq1[:], ps1[:], ACTF.Square)
                        nc.scalar.activation(sq2[:], ps2[:], ACTF.Square)
                        psvar = psA1.tile([1, TCH], F32, tag="t_aux")
                        nc.tensor.matmul(psvar[:], onesc[:], sq1[:], start=True, stop=False)
                        nc.tensor.matmul(psvar[:], onesc[:], sq2[:], start=False, stop=True)
                        C1, S1, C2, S2 = (tabt[k] for k in tkeys)
                        m1 = sa.tile([128, TCH], F32, tag="m1")
                        m2 = sa.tile([128, TCH], F32, tag="m2")
                        m3 = sa.tile([128, TCH], F32, tag="m1", name="m3t")
                        m4 = sa.tile([128, TCH], F32, tag="m2", name="m4t")
                        if qhead is None:
                            # k: apply rstd via PE broadcast, fused into rope
                            stdv = sa.tile([1, TCH], F32R, tag="stdv")
                            nc.scalar.activation(stdv[:], psvar[:], ACTF.Sqrt,
                                                 scale=1.0 / HEAD_DIM, bias=epsb[:])
                            psb = psA1.tile([128, TCH], F32, tag="t_aux")
                            nc.tensor.matmul(psb[:], ones1[:], stdv[:],
                                             start=True, stop=True)
                            rb = sa.tile([128, TCH], F32, tag="rb")
                            nc.vector.reciprocal_approx_fast(out=rb[:], in_=psb[:])
                            a1 = sa.tile([128, TCH], F32, tag="a1")
                            a2 = sa.tile([128, TCH], F32, tag="a2")
                            nc.vector.tensor_tensor(a1[:], ps1[:], rb[:], AluOpType.mult)
                            nc.vector.tensor_tensor(a2[:], ps2[:], rb[:], AluOpType.mult)
                        else:
                            # q: defer 1/std to the stage-B logits scale;
                            # transpose 16*std per 128-tile via K=1 matmuls
                            stdvf = sa.tile([1, TCH], F32, tag="stdvf")
                            nc.scalar.activation(stdvf[:], psvar[:], ACTF.Sqrt,
                                                 bias=epsbq[:])
                            sq_ps = psA1.tile([128, TCH // 128], F32, tag="t_aux")
                            for s in range(TCH // 128):
                                nc.tensor.matmul(
                                    sq_ps[:, s:s + 1],
                                    stdvf[:, s * 128:(s + 1) * 128],
                                    ones_f[:], start=True, stop=True)
                            stdq = sa.tile([128, TCH // 128], F32, tag="stdq")
                            nc.scalar.activation(stdq[:], sq_ps[:], ACTF.Copy)
                            nc.vector.reciprocal_approx_fast(
                                out=rstdq_c[qhead][:, tci * (TCH // 128):
                                                   (tci + 1) * (TCH // 128)],
                                in_=stdq[:])
                            a1, a2 = ps1, ps2
                        nc.vector.tensor_tensor(m1[:], a1[:], C1[:], AluOpType.mult)
                        nc.vector.tensor_tensor(m2[:], a2[:], S2[:], AluOpType.mult)
                        nc.vector.tensor_tensor(
                            dest[dbase][:, t0:t0 + TCH], m1[:], m2[:], AluOpType.subtract)
                        nc.vector.tensor_tensor(m3[:], a2[:], C2[:], AluOpType.mult)
                        nc.vector.tensor_tensor(m4[:], a1[:], S1[:], AluOpType.mult)
                        nc.vector.tensor_tensor(
                            dest[dbase + 1][:, t0:t0 + TCH], m3[:], m4[:], AluOpType.add)
                    # v projection: transposed matmuls (N=512), PE-transpose back
                    vT_sb = sa.tile([128, 2 * TCH], F32R, tag="vTsb")
                    for cc in range(2):
                        psv = psA.tile([128, TCH], F32, tag="t_eps")
                        for wc in range(NW):
                            nc.tensor.matmul(
                                psv[:],
                                wv_t[:, wc * 256 + cc * 128: wc * 256 + (cc + 1) * 128],
                                xts[:, wc * TCH:(wc + 1) * TCH],
                                start=(wc == 0), stop=(wc == NW - 1),
                            )
                        nc.scalar.activation(vT_sb[:, cc * TCH:(cc + 1) * TCH],
                                             psv[:], ACTF.Copy)
                    for s in range(TCH // 128):
                        vsb = sa.tile([128, 256], F32R, tag="vsb")
                        for cc in range(2):
                            psvt = psA1.tile([128, 128], F32R, tag="t_aux")
                            nc.tensor.transpose(
                                psvt[:],
                                vT_sb[:, cc * TCH + s * 128: cc * TCH + (s + 1) * 128],
                                ident[:])
                            nc.vector.tensor_copy(vsb[:, cc * 128:(cc + 1) * 128],
                                                  psvt[:].bitcast(F32))
                        nc.sync.dma_start(
                            out=v_dram[t0 + s * 128: t0 + (s + 1) * 128, :], in_=vsb[:])
                        if debug_taps:
                            nc.sync.dma_start(
                                out=taps["v_tap"][t0 + s * 128: t0 + (s + 1) * 128, :],
                                in_=vsb[:].bitcast(F32))
            if debug_taps:
                for c in range(4):
                    nc.sync.dma_start(out=taps["qT_tap"][c * 128:(c + 1) * 128, :],
                                      in_=qT[c][:].bitcast(F32))
                for c in range(2):
                    nc.sync.dma_start(out=taps["kT_tap"][c * 128:(c + 1) * 128, :],
                                      in_=kT[c][:].bitcast(F32))

            # ---------------- stage B: banded attention ----------------------
            with (
                tc.tile_pool(name="encp", bufs=1) as encp,
                tc.tile_pool(name="woutp", bufs=1) as woutp,
            ):
                encT = [encp.tile([128, T], F32R, tag=f"encT{c}", name=f"encT{c}") for c in range(4)]
                wout_t = [woutp.tile([128, T], F32R, tag=f"wo{c}", name=f"wo{c}") for c in range(4)]
                wout_r = wout.rearrange("(c p) t -> c p t", p=128)
                for c in range(4):
                    nc.sync.dma_start(out=wout_t[c][:], in_=wout_r[c])

                with (
                    tc.tile_pool(name="sb", bufs=2) as sbp,
                    tc.tile_pool(name="vstage", bufs=2) as vsp,
                    tc.tile_pool(name="ptp", bufs=2) as ptp,
                ):
                    def emit_pv(g, ginfo, vt_all, pts_all):
                        for i, (jc, inA, inB) in enumerate(ginfo):
                            for h in range(2):
                                if not inA:
                                    nc.vector.tensor_copy(
                                        pts_all[:, i * 512 + h * 256:
                                                i * 512 + h * 256 + 128], zero_r[:])
                                if not inB:
                                    nc.vector.tensor_copy(
                                        pts_all[:, i * 512 + h * 256 + 128:
                                                i * 512 + h * 256 + 256], zero_r[:])
                        for cc in range(2):
                            eps = psE.tile([128, 512], F32, tag="t_eps", name=f"eps{g}_{cc}")
                            for i, (jc, _, _) in enumerate(ginfo):
                                nc.tensor.matmul(
                                    eps[:], vt_all[:, i * 256 + cc * 128: i * 256 + (cc + 1) * 128],
                                    pts_all[:, i * 512:(i + 1) * 512],
                                    start=(i == 0), stop=(i == len(ginfo) - 1),
                                )
                            for h in range(2):
                                nc.scalar.activation(
                                    encT[2 * h + cc][:, g * 256:(g + 1) * 256],
                                    eps[:, h * 256:(h + 1) * 256], ACTF.Copy)

                    pdict = {}
                    pending = None

                    def emit_group(g):
                        nonlocal pending
                        ginfo = groups[g]
                        nj = len(ginfo)
                        jc0 = ginfo[0][0]
                        vt_all = vsp.tile([128, nj * 256], F32R, tag="vt", name=f"vt{g}")
                        nc.sync.dma_start(
                            out=vt_all[:].rearrange("p (c m) -> p c m", m=256),
                            in_=v_dram.rearrange("(c p) m -> p c m", p=128)[:, jc0:jc0 + nj])
                        pts_all = ptp.tile([128, nj * 512], F32R, tag="pts",
                                           name=f"pts{g}")
                        den2 = sbp.tile([128, 2 * 2], F32, tag="den2", name=f"den2_{g}")
                        for half in range(2):
                            it = 2 * g + half
                            jst = js[it]
                            bias_t = sbp.tile([128, Wb], BF16, tag="bias", name=f"bias{it}")
                            nc.sync.dma_start(out=bias_t[:], in_=bias_d[it])
                            for h in range(2):
                                S_pieces = [
                                    psS.tile([128, pw], F32, tag=(f"t_s{pi}" if pi < 2 else "t_aux"),
                                             name=f"S{it}_{h}_{pi}")
                                    for pi, pw in enumerate(pieces)]
                                for cc in range(2):
                                    col = 0
                                    for pi, pw in enumerate(pieces):
                                        nc.tensor.matmul(
                                            S_pieces[pi][:],
                                            qT[2 * h + cc][:, it * 128:(it + 1) * 128],
                                            kT[cc][:, jst + col: jst + col + pw],
                                            start=(cc == 0), stop=(cc == 1),
                                        )
                                        col += pw
                                S_b = sbp.tile([128, Wb], F32, tag="Sb", name=f"Sb{it}_{h}")
                                col = 0
                                for pi, pw in enumerate(pieces):
                                    nc.vector.scalar_tensor_tensor(
                                        S_b[:, col:col + pw], S_pieces[pi][:],
                                        rstdq_c[h][:, it:it + 1],
                                        bias_t[:, col:col + pw],
                                        AluOpType.mult, AluOpType.add)
                                    col += pw
                                P_t = sbp.tile([128, Wb], F32, tag=f"P{h}", name=f"P{it}_{h}")
                                nc.scalar.activation(
                                    P_t[:], S_b[:], ACTF.Exp,
                                    accum_out=den2[:, half * 2 + h: half * 2 + h + 1])
                                pdict[(half, h)] = P_t
                            rden = sbp.tile([128, 2], F32, tag="rden", name=f"rden{it}")
                            nc.vector.reciprocal_approx_fast(
                                out=rden[:], in_=den2[:, half * 2: half * 2 + 2])
                            for h in range(2):
                                P_t = pdict[(half, h)]
                                Pn = sbp.tile([128, Wb], F32R, tag="Pn", name=f"Pn{it}_{h}")
                                nc.vector.tensor_scalar_mul(Pn[:], P_t[:], rden[:, h:h + 1])
                                idx0 = next(i for i, (c, _, _) in enumerate(ginfo)
                                            if c == jst // 128)
                                pts_v = pts_all[:].rearrange(
                                    "p (i f c) -> p i f c", f=4, c=128)
                                lj = 0
                                while lj < NJ:
                                    nb = min(3, NJ - lj)
                                    ps_t = psT.tile([128, 3 * 128], F32R, tag="t_aux",
                                                    name=f"ptps{it}_{h}_{lj}")
                                    for k in range(nb):
                                        nc.tensor.transpose(
                                            ps_t[:, k * 128:(k + 1) * 128],
                                            Pn[:, (lj + k) * 128:(lj + k + 1) * 128],
                                            ident[:])
                                    nc.vector.tensor_copy(
                                        pts_v[:, idx0 + lj: idx0 + lj + nb,
                                              h * 2 + half, :],
                                        ps_t[:, 0:nb * 128].bitcast(F32).rearrange(
                                            "p (k c) -> p k c", c=128))
                                    lj += nb
                        if pending is not None:
                            emit_pv(*pending)
                        pending = (g, ginfo, vt_all, pts_all)

                    for g in range(NT // 2):
                        if g < 2:
                            with tc.high_priority(offset=330):
                                emit_group(g)
                        else:
                            emit_group(g)
                    emit_pv(*pending)

                if debug_taps:
                    for c in range(4):
                        nc.sync.dma_start(
                            out=taps["encT_tap"][c * 128:(c + 1) * 128, :],
                            in_=encT[c][:].bitcast(F32))
                ps_shared.__exit__(None, None, None)

                # ---------------- stage C: output projection ----------------------
                with (
                    tc.tile_pool(name="outp", bufs=2) as outp,
                    tc.tile_pool(name="psO", bufs=2, space="PSUM") as psO,
                ):
                    for tt in range(NT):
                        ops = psO.tile([128, T], F32, tag="ops", name=f"ops{tt}")
                        for cc in range(4):
                            for nb in range(4):
                                nc.tensor.matmul(
                                    ops[:, nb * 512:(nb + 1) * 512],
                                    encT[cc][:, tt * 128:(tt + 1) * 128],
                                    wout_t[cc][:, nb * 512:(nb + 1) * 512],
                                    start=(cc == 0), stop=(cc == 3),
                                )
                        ob = outp.tile([128, T], F32, tag="ob", name=f"ob{tt}")
                        nc.scalar.activation(ob[:], ops[:], ACTF.Copy)
                        for nb in range(4):
                            nc.sync.dma_start(
                                out=yp[tt * 128:(tt + 1) * 128,
                                       nb * 512:(nb + 1) * 512],
                                in_=ob[:, nb * 512:(nb + 1) * 512])

    nc.compile()
    return nc


def kernel(x, positions, attn_mask, wq, wkv, wout, q_scale, k_scale):
    x = np.ascontiguousarray(x, np.float32)
    positions = np.asarray(positions)
    wq = np.ascontiguousarray(wq, np.float32)
    wkv = np.ascontiguousarray(wkv, np.float32)
    wout = np.ascontiguousarray(wout, np.float32)
    q_scale = np.asarray(q_scale, np.float32)
    k_scale = np.asarray(k_scale, np.float32)

    valid, Wb, js = _geometry(positions, attn_mask)
    shared = not (q_scale.any() or k_scale.any())

    key = (Wb, js, shared, DEBUG_TAPS)
    if key not in _prog_cache:
        _prog_cache[key] = _build(Wb, js, shared, DEBUG_TAPS)
    nc = _prog_cache[key]

    # host-side bias bands: 0 where valid, MASK_NEG elsewhere (incl. padding)
    bias = np.full((B, NT, 128, Wb), MASK_NEG, np.float32)
    for it in range(NT):
        j0 = js[it]
        w = min(Wb, T - j0)
        vslab = valid[:, it * 128:(it + 1) * 128, j0:j0 + w]
        bias[:, it, :, :w][vslab] = 0.0
    bias = bias.astype(ml_dtypes.bfloat16)

    ident = np.eye(128, dtype=np.float32)
    ones1 = np.ones((1, 128), np.float32)
    onesc = np.ones((128, 1), np.float32)

    in_maps = []
    for core in range(8):
        b, kh = divmod(core, NUM_KV_HEADS)
        m = {
            "xT": np.ascontiguousarray(x[b].T),
            "wq": np.ascontiguousarray(wq[:, kh * 512:(kh + 1) * 512]),
            "wk": np.ascontiguousarray(wkv[:, kh * 256:(kh + 1) * 256]),
            "wv": np.ascontiguousarray(wkv[:, 1024 + kh * 256: 1024 + (kh + 1) * 256]),
            "wout": np.ascontiguousarray(wout[kh * 512:(kh + 1) * 512, :]),
            "ident": ident, "ones1": ones1, "onesc": onesc,
            "bias": bias[b],
        }
        if shared:
            ct, st, _, _ = _rope_tables(positions[b], np.zeros(HEAD_DIM, np.float32))
            m["ct"], m["st"] = ct, st
        else:
            for nm, tb in zip(("cq1", "sq1", "cq2", "sq2"),
                              _rope_tables(positions[b], q_scale)):
                m[nm] = tb
            for nm, tb in zip(("ck1", "sk1", "ck2", "sk2"),
                              _rope_tables(positions[b], k_scale)):
                m[nm] = tb
        in_maps.append(m)

    res = run_bass_kernel_spmd(nc, in_maps, list(range(8)))
    kernel._last_results = res
    out = np.empty((B, T, T), np.float32)
    for b in range(B):
        acc = res.results[b * NUM_KV_HEADS]["yp"].astype(np.float64)
        for kh in range(1, NUM_KV_HEADS):
            acc += res.results[b * NUM_KV_HEADS + kh]["yp"]
        out[b] = acc.astype(np.float32)
    return out



# revision 15
# speedup vs baseline: 1.3672x; 1.3672x over previous
"""Trainium2 Bass kernel for nn_Attention_83081847374268 (sparse sliding-window GQA).

Sharding: 8 cores = batch (2, data parallel) x kv-head (4, tensor parallel).
Each core computes, for its (b, kh): q/k/v projections (2 q heads, 1 kv head),
QK-RMSNorm + RoPE, banded sliding-window attention, and a partial output
projection against its 512-row slice of wout.  The host sums the 4 partials
per batch (the TP reduction) and stacks the batches.

Device dataflow (per core):
  stage A: stream xT column-chunks; matmul projections directly in transposed
           layout (qT/kT [head_dim, T]); RMSNorm via ones-matmul variance +
           PE-broadcast rstd; RoPE fused with the rstd multiply on DVE.
           v computed in natural layout [T, 256] and bounced via DRAM.
  stage B: per 128-query tile: S = qT.T @ kT over a host-chosen key window,
           additive mask bias (host-precomputed, handles any attn_mask /
           positions pattern), exp on ACT with fused row-sum, normalize on
           DVE, PE-transpose P, PV matmuls accumulate encoded^T.
  stage C: out partial = encT.T @ wout_slice, DMA per 128-row tile.

All matmuls run as float32r (TRN2 rounds fp32r operands to 12 mantissa bits
on write; full speed at moving-dim >= 256).
"""
import sys

sys.path.insert(0, "/opt/trn_rl_repo")

import numpy as np
import ml_dtypes

import concourse.bacc as bacc
import concourse.mybir as mybir
from concourse.bass_utils import run_bass_kernel_spmd
from concourse.tile import TileContext
from concourse.alu_op_type import AluOpType

F32 = mybir.dt.float32
F32R = mybir.dt.float32r
BF16 = mybir.dt.bfloat16
F16 = mybir.dt.float16
ACTF = mybir.ActivationFunctionType

B, T, WIDTH = 2, 2048, 2048
NUM_HEADS, NUM_KV_HEADS, HEAD_DIM = 8, 4, 256
GROUPS = NUM_HEADS // NUM_KV_HEADS  # 2 q heads per kv head (= per core)
WINDOW = 512
ROPE_BASE = 10000.0
ALPHA = HEAD_DIM ** -0.5
MASK_NEG = -100.0  # exp(S/16 + MASK_NEG) == 0 for |S|<=~16; exact in bf16

NT = T // 128           # 16 query tiles
TCH = 512               # stage-A t-chunk width
NTCH = T // TCH         # 4
NW = WIDTH // 128       # 16 contraction chunks

_prog_cache = {}
DEBUG_TAPS = False


def _round_up(x, m):
    return (x + m - 1) // m * m


def _geometry(positions, attn_mask):
    """Per-query-tile key windows from the actual mask/positions data."""
    pos = np.asarray(positions)
    am = np.asarray(attn_mask)
    pd = pos[:, :, None].astype(np.int64) - pos[:, None, :].astype(np.int64)
    valid = am & (np.abs(pd) < WINDOW)  # [B, T, T] bool
    assert valid.any(axis=2).all(), "a query row with no valid key is unsupported"
    js = []
    wmax = 0
    for it in range(NT):
        cols = valid[:, it * 128:(it + 1) * 128, :].any(axis=(0, 1))
        idx = np.nonzero(cols)[0]
        j_lo, j_hi = int(idx[0]), int(idx[-1]) + 1
        j0 = (j_lo // 128) * 128
        wmax = max(wmax, j_hi - j0)
        js.append(j0)
    Wb = max(256, _round_up(wmax, 128))
    Wb = min(Wb, T)
    js = tuple(max(0, min(j, T - Wb)) for j in js)
    return valid, Wb, js


def _pieces(Wb):
    """Split Wb into moving-dim pieces from {512, 384, 256} (fp32r full speed
    needs N >= 256; Wb is a multiple of 128 and >= 256)."""
    out = []
    rem = Wb
    while rem > 640:
        out.append(512)
        rem -= 512
    if rem == 128:
        out[-1] = 384
        rem = 256
    if rem > 512:
        out.append(384)
        rem -= 384
    if rem:
        out.append(rem)
    return out


def _rope_tables(pos_b, scale):
    """cos/sin tables in [head_dim/2, T] (transposed) layout, gain folded in."""
    d = np.arange(HEAD_DIM // 2, dtype=np.float32)
    timescale = (ROPE_BASE ** (2.0 / HEAD_DIM * d)).astype(np.float32)
    rad = pos_b.astype(np.float32)[None, :] / timescale[:, None]  # [128, T]
    cos, sin = np.cos(rad).astype(np.float32), np.sin(rad).astype(np.float32)
    g1 = (1.0 + scale[:HEAD_DIM // 2]).astype(np.float32)[:, None]
    g2 = (1.0 + scale[HEAD_DIM // 2:]).astype(np.float32)[:, None]
    # o1 = a1*C1 - a2*S2 ; o2 = a2*C2 + a1*S1
    return (cos * g1, sin * g1, cos * g2, sin * g2)  # C1, S1, C2, S2


def _build(Wb, js, shared_tables, debug_taps=False):
    nc = bacc.Bacc("TRN2", target_bir_lowering=False, debug=False, num_devices=8)

    def din(name, shape, dt):
        return nc.dram_tensor(name, shape, dt, kind="ExternalInput").ap()

    xT = din("xT", [WIDTH, T], BF16)
    wq = din("wq", [WIDTH, 512], BF16)
    wk = din("wk", [WIDTH, 256], BF16)
    wv = din("wv", [WIDTH, 256], BF16)
    wout = din("wout", [512, T], BF16)
    ident_d = din("ident", [128, 128], BF16)
    ones1_d = din("ones1", [1, 128], F32R)    # K=1 broadcast lhsT
    onesc_d = din("onesc", [128, 1], F32R)    # partition-sum lhsT
    bias_d = din("bias", [NT, 128, Wb], BF16)
    tab_names = ["ct", "st"] if shared_tables else [
        "cq1", "sq1", "cq2", "sq2", "ck1", "sk1", "ck2", "sk2"]
    tabs = {n: din(n, [128, T], BF16) for n in tab_names}
    yp = nc.dram_tensor("yp", [T, T], F16, kind="ExternalOutput").ap()

    taps = {}
    if debug_taps:
        for nm, shape in (("qT_tap", [512, T]), ("kT_tap", [256, T]),
                          ("v_tap", [T, 256]), ("encT_tap", [512, T])):
            taps[nm] = nc.dram_tensor(nm, shape, F32, kind="ExternalOutput").ap()

    pieces = _pieces(Wb)
    NJ = Wb // 128  # P-transpose blocks per tile
    # per-group (2 query tiles) union of key chunks, and which halves exist
    groups = []
    for g in range(NT // 2):
        w0 = set(range(js[2 * g] // 128, js[2 * g] // 128 + NJ))
        w1 = set(range(js[2 * g + 1] // 128, js[2 * g + 1] // 128 + NJ))
        groups.append([(jc, jc in w0, jc in w1) for jc in sorted(w0 | w1)])

    with TileContext(nc) as tc:
        with (
            tc.tile_pool(name="persist", bufs=1) as pp,
            tc.tile_pool(name="qk_store", bufs=1) as qkp,
        ):
            ident = pp.tile([128, 128], BF16)
            nc.sync.dma_start(out=ident[:], in_=ident_d[:])
            ones1 = pp.tile([1, 128], F32R)
            nc.sync.dma_start(out=ones1[:], in_=ones1_d[:])
            onesc = pp.tile([128, 1], F32R)
            nc.sync.dma_start(out=onesc[:], in_=onesc_d[:])
            epsb = pp.tile([1, 1], F32)
            nc.any.memset(epsb[:], 1e-6)
            epsbq = pp.tile([1, 1], F32)
            nc.any.memset(epsbq[:], HEAD_DIM * 1e-6)
            ones_f = pp.tile([1, 1], F32)
            nc.any.memset(ones_f[:], 1.0)
            rstdq_c = [pp.tile([128, NT], F32, tag=f"rstdq{hh}", name=f"rstdq{hh}")
                       for hh in range(2)]
            zrow_f = pp.tile([128, 128], F32)
            nc.any.memset(zrow_f[:], 0.0)
            zero_r = pp.tile([128, 128], BF16)
            nc.vector.tensor_copy(zero_r[:], zrow_f[:])

            qT = [qkp.tile([128, T], BF16, tag=f"qT{c}", name=f"qT{c}") for c in range(4)]
            kT = [qkp.tile([128, T], BF16, tag=f"kT{c}", name=f"kT{c}") for c in range(2)]
            # v resident in SBUF, natural [keys, head_dim] layout per 128-chunk
            vt = qkp.tile([128, NT * 256], BF16, tag="vt", name="vt")

            # ---------------- stage A: projections + RMSNorm + RoPE ----------
            ps_shared = tc.tile_pool(name="ps_shared", bufs=2, space="PSUM")
            psA = psA1 = psS = psT = psE = ps_shared.__enter__()
            with (
                tc.tile_pool(name="wpool", bufs=1) as wp,
                tc.tile_pool(name="xpool", bufs=2) as xp,
                tc.tile_pool(name="tabpool", bufs=1) as tp,
                tc.tile_pool(name="sa", bufs=1) as sa,
            ):
                wq_t = wp.tile([128, NW * 512], BF16)
                wk_t = wp.tile([128, NW * 256], BF16)
                wv_t = wp.tile([128, NW * 256], BF16)
                wq_r = wq.rearrange("(c p) m -> p c m", p=128)
                wk_r = wk.rearrange("(c p) m -> p c m", p=128)
                wv_r = wv.rearrange("(c p) m -> p c m", p=128)
                wq_v = wq_t[:].rearrange("p (c m) -> p c m", m=512)
                wk_v = wk_t[:].rearrange("p (c m) -> p c m", m=256)
                wv_v = wv_t[:].rearrange("p (c m) -> p c m", m=256)
                xT_r = xT.rearrange("(c p) t -> p c t", p=128)

                def load_xts(tci):
                    t0 = tci * TCH
                    xts = xp.tile([128, NW * TCH], BF16, tag="xts", name=f"xts{tci}")
                    xv = xts[:].rearrange("p (c t) -> p c t", t=TCH)
                    for q4 in range(4):
                        nc.sync.dma_start(
                            out=xv[:, q4 * 4:(q4 + 1) * 4],
                            in_=xT_r[:, q4 * 4:(q4 + 1) * 4, t0:t0 + TCH],
                        )
                    return xts

                xts_pre = xp.tile([128, NW * TCH], BF16, tag="xts", name="xts0")
                xv0 = xts_pre[:].rearrange("p (c t) -> p c t", t=TCH)
                for q4 in range(4):
                    nc.sync.dma_start(out=wk_v[:, q4 * 4:(q4 + 1) * 4],
                                      in_=wk_r[:, q4 * 4:(q4 + 1) * 4])
                    nc.sync.dma_start(out=xv0[:, q4 * 4:(q4 + 1) * 4],
                                      in_=xT_r[:, q4 * 4:(q4 + 1) * 4, 0:TCH])
                for q4 in range(4):
                    nc.sync.dma_start(out=wq_v[:, q4 * 4:(q4 + 1) * 4],
                                      in_=wq_r[:, q4 * 4:(q4 + 1) * 4])

                if shared_tables:
                    q_tabs = k_tabs = ("ct", "st", "ct", "st")
                else:
                    q_tabs = ("cq1", "sq1", "cq2", "sq2")
                    k_tabs = ("ck1", "sk1", "ck2", "sk2")
                units = [
                    (wk_t, 256, 0, k_tabs, kT, 0, None),
                    (wq_t, 512, 0, q_tabs, qT, 0, 0),
                    (wq_t, 512, 256, q_tabs, qT, 2, 1),
                ]
                for tci in range(NTCH):
                    t0 = tci * TCH
                    xts = xts_pre if tci == 0 else load_xts(tci)
                    # q/k projections in transposed layout + norm + rope
                    tabt = {}
                    for name in dict.fromkeys(q_tabs + k_tabs):
                        tt = tp.tile([128, TCH], BF16, tag=name, name=f"tab_{name}")
                        nc.sync.dma_start(out=tt[:], in_=tabs[name][:, t0:t0 + TCH])
                        tabt[name] = tt
                    if tci == 0:
                        for q4 in range(4):
                            nc.sync.dma_start(out=wv_v[:, q4 * 4:(q4 + 1) * 4],
                                              in_=wv_r[:, q4 * 4:(q4 + 1) * 4])
                    for w_t, wcols, cbase, tkeys, dest, dbase, qhead in units:
                        ps1 = psA.tile([128, TCH], F32, tag="t_s0")
                        ps2 = psA.tile([128, TCH], F32, tag="t_s1")
                        for ps, cc in ((ps1, 0), (ps2, 1)):
                            coff = cbase + cc * 128
                            for wc in range(NW):
                                nc.tensor.matmul(
                                    ps[:],
                                    w_t[:, wc * wcols + coff: wc * wcols + coff + 128],
                                    xts[:, wc * TCH:(wc + 1) * TCH],
                                    start=(wc == 0), stop=(wc == NW - 1),
                                )
                        sq1 = sa.tile([128, TCH], F32R, tag="sq1")
                        sq2 = sa.tile([128, TCH], F32R, tag="sq2")
                        nc.scalar.activation(sq1[:], ps1[:], ACTF.Square)
                        nc.scalar.activation(sq2[:], ps2[:], ACTF.Square)
                        psvar = psA1.tile([1, TCH], F32, tag="t_aux")
                        nc.tensor.matmul(psvar[:], onesc[:], sq1[:], start=True, stop=False)
                        nc.tensor.matmul(psvar[:], onesc[:], sq2[:], start=False, stop=True)
                        C1, S1, C2, S2 = (tabt[k] for k in tkeys)
                        m1 = sa.tile([128, TCH], F32, tag="m1")
                        m2 = sa.tile([128, TCH], F32, tag="m2")
                        m3 = sa.tile([128, TCH], F32, tag="m1", name="m3t")
                        m4 = sa.tile([128, TCH], F32, tag="m2", name="m4t")
                        if qhead is None:
                            # k: apply rstd via PE broadcast, fused into rope
                            stdv = sa.tile([1, TCH], F32R, tag="stdv")
                            nc.scalar.activation(stdv[:], psvar[:], ACTF.Sqrt,
                                                 scale=1.0 / HEAD_DIM, bias=epsb[:])
                            psb = psA1.tile([128, TCH], F32, tag="t_aux")
                            nc.tensor.matmul(psb[:], ones1[:], stdv[:],
                                             start=True, stop=True)
                            rb = sa.tile([128, TCH], F32, tag="rb")
                            nc.vector.reciprocal_approx_fast(out=rb[:], in_=psb[:])
                            a1 = sa.tile([128, TCH], F32, tag="a1")
                            a2 = sa.tile([128, TCH], F32, tag="a2")
                            nc.vector.tensor_tensor(a1[:], ps1[:], rb[:], AluOpType.mult)
                            nc.vector.tensor_tensor(a2[:], ps2[:], rb[:], AluOpType.mult)
                        else:
                            # q: defer 1/std to the stage-B logits scale;
                            # transpose 16*std per 128-tile via K=1 matmuls
                            stdvf = sa.tile([1, TCH], F32, tag="stdvf")
                            nc.scalar.activation(stdvf[:], psvar[:], ACTF.Sqrt,
                                                 bias=epsbq[:])
                            sq_ps = psA1.tile([128, TCH // 128], F32, tag="t_aux")
                            for s in range(TCH // 128):
                                nc.tensor.matmul(
                                    sq_ps[:, s:s + 1],
                                    stdvf[:, s * 128:(s + 1) * 128],
                                    ones_f[:], start=True, stop=True)
                            stdq = sa.tile([128, TCH // 128], F32, tag="stdq")
                            nc.scalar.activation(stdq[:], sq_ps[:], ACTF.Copy)
                            nc.vector.reciprocal_approx_fast(
                                out=rstdq_c[qhead][:, tci * (TCH // 128):
                                                   (tci + 1) * (TCH // 128)],
                                in_=stdq[:])
                            a1, a2 = ps1, ps2
                        nc.vector.tensor_tensor(m1[:], a1[:], C1[:], AluOpType.mult)
                        nc.vector.tensor_tensor(m2[:], a2[:], S2[:], AluOpType.mult)
                        nc.vector.tensor_tensor(
                            dest[dbase][:, t0:t0 + TCH], m1[:], m2[:], AluOpType.subtract)
                        nc.vector.tensor_tensor(m3[:], a2[:], C2[:], AluOpType.mult)
                        nc.vector.tensor_tensor(m4[:], a1[:], S1[:], AluOpType.mult)
                        nc.vector.tensor_tensor(
                            dest[dbase + 1][:, t0:t0 + TCH], m3[:], m4[:], AluOpType.add)
                    # v projection: transposed matmuls (N=512), PE-transpose into
                    # the persistent SBUF tile (natural [keys, head_dim] layout)
                    vT_sb = sa.tile([128, 2 * TCH], BF16, tag="vTsb")
                    for cc in range(2):
                        psv = psA.tile([128, TCH], F32, tag="t_eps")
                        for wc in range(NW):
                            nc.tensor.matmul(
                                psv[:],
                                wv_t[:, wc * 256 + cc * 128: wc * 256 + (cc + 1) * 128],
                                xts[:, wc * TCH:(wc + 1) * TCH],
                                start=(wc == 0), stop=(wc == NW - 1),
                            )
                        nc.scalar.activation(vT_sb[:, cc * TCH:(cc + 1) * TCH],
                                             psv[:], ACTF.Copy)
                    for s in range(TCH // 128):
                        jc = tci * (TCH // 128) + s
                        for cc in range(2):
                            psvt = psA1.tile([128, 128], BF16, tag="t_aux")
                            nc.tensor.transpose(
                                psvt[:],
                                vT_sb[:, cc * TCH + s * 128: cc * TCH + (s + 1) * 128],
                                ident[:])
                            nc.vector.tensor_copy(
                                vt[:, jc * 256 + cc * 128: jc * 256 + (cc + 1) * 128],
                                psvt[:])
            if debug_taps:
                for c in range(4):
                    nc.sync.dma_start(out=taps["qT_tap"][c * 128:(c + 1) * 128, :],
                                      in_=qT[c][:].bitcast(F32))
                for c in range(2):
                    nc.sync.dma_start(out=taps["kT_tap"][c * 128:(c + 1) * 128, :],
                                      in_=kT[c][:].bitcast(F32))

            # ---------------- stage B: banded attention ----------------------
            with (
                tc.tile_pool(name="encp", bufs=1) as encp,
                tc.tile_pool(name="woutp", bufs=1) as woutp,
            ):
                encT = [encp.tile([128, T], BF16, tag=f"encT{c}", name=f"encT{c}") for c in range(4)]
                wout_t = [woutp.tile([128, T], BF16, tag=f"wo{c}", name=f"wo{c}") for c in range(4)]
                wout_r = wout.rearrange("(c p) t -> c p t", p=128)
                for c in range(4):
                    nc.sync.dma_start(out=wout_t[c][:], in_=wout_r[c])

                with (
                    tc.tile_pool(name="sb", bufs=2) as sbp,
                    tc.tile_pool(name="ptp", bufs=2) as ptp,
                ):
                    def emit_pv(g, ginfo, pts_all):
                        jc0 = ginfo[0][0]
                        for i, (jc, inA, inB) in enumerate(ginfo):
                            for h in range(2):
                                if not inA:
                                    nc.vector.tensor_copy(
                                        pts_all[:, i * 512 + h * 256:
                                                i * 512 + h * 256 + 128], zero_r[:])
                                if not inB:
                                    nc.vector.tensor_copy(
                                        pts_all[:, i * 512 + h * 256 + 128:
                                                i * 512 + h * 256 + 256], zero_r[:])
                        for cc in range(2):
                            eps = psE.tile([128, 512], F32, tag="t_eps", name=f"eps{g}_{cc}")
                            for i, (jc, _, _) in enumerate(ginfo):
                                nc.tensor.matmul(
                                    eps[:],
                                    vt[:, jc * 256 + cc * 128: jc * 256 + (cc + 1) * 128],
                                    pts_all[:, i * 512:(i + 1) * 512],
                                    start=(i == 0), stop=(i == len(ginfo) - 1),
                                )
                            for h in range(2):
                                nc.scalar.activation(
                                    encT[2 * h + cc][:, g * 256:(g + 1) * 256],
                                    eps[:, h * 256:(h + 1) * 256], ACTF.Copy)

                    pdict = {}
                    pending = None

                    def emit_group(g):
                        nonlocal pending
                        ginfo = groups[g]
                        nj = len(ginfo)
                        jc0 = ginfo[0][0]
                        pts_all = ptp.tile([128, nj * 512], BF16, tag="pts",
                                           name=f"pts{g}")
                        den2 = sbp.tile([128, 2 * 2], F32, tag="den2", name=f"den2_{g}")
                        for half in range(2):
                            it = 2 * g + half
                            jst = js[it]
                            bias_t = sbp.tile([128, Wb], BF16, tag="bias", name=f"bias{it}")
                            nc.sync.dma_start(out=bias_t[:], in_=bias_d[it])
                            for h in range(2):
                                S_pieces = [
                                    psS.tile([128, pw], F32, tag=(f"t_s{pi}" if pi < 2 else "t_aux"),
                                             name=f"S{it}_{h}_{pi}")
                                    for pi, pw in enumerate(pieces)]
                                for cc in range(2):
                                    col = 0
                                    for pi, pw in enumerate(pieces):
                                        nc.tensor.matmul(
                                            S_pieces[pi][:],
                                            qT[2 * h + cc][:, it * 128:(it + 1) * 128],
                                            kT[cc][:, jst + col: jst + col + pw],
                                            start=(cc == 0), stop=(cc == 1),
                                        )
                                        col += pw
                                S_b = sbp.tile([128, Wb], F32, tag="Sb", name=f"Sb{it}_{h}")
                                col = 0
                                for pi, pw in enumerate(pieces):
                                    nc.vector.scalar_tensor_tensor(
                                        S_b[:, col:col + pw], S_pieces[pi][:],
                                        rstdq_c[h][:, it:it + 1],
                                        bias_t[:, col:col + pw],
                                        AluOpType.mult, AluOpType.add)
                                    col += pw
                                P_t = sbp.tile([128, Wb], F32, tag=f"P{h}", name=f"P{it}_{h}")
                                nc.scalar.activation(
                                    P_t[:], S_b[:], ACTF.Exp,
                                    accum_out=den2[:, half * 2 + h: half * 2 + h + 1])
                                pdict[(half, h)] = P_t
                            rden = sbp.tile([128, 2], F32, tag="rden", name=f"rden{it}")
                            nc.vector.reciprocal_approx_fast(
                                out=rden[:], in_=den2[:, half * 2: half * 2 + 2])
                            for h in range(2):
                                P_t = pdict[(half, h)]
                                Pn = sbp.tile([128, Wb], BF16, tag="Pn", name=f"Pn{it}_{h}")
                                nc.vector.tensor_scalar_mul(Pn[:], P_t[:], rden[:, h:h + 1])
                                idx0 = next(i for i, (c, _, _) in enumerate(ginfo)
                                            if c == jst // 128)
                                pts_v = pts_all[:].rearrange(
                                    "p (i f c) -> p i f c", f=4, c=128)
                                lj = 0
                                while lj < NJ:
                                    nb = min(3, NJ - lj)
                                    ps_t = psT.tile([128, 3 * 128], BF16, tag="t_aux",
                                                    name=f"ptps{it}_{h}_{lj}")
                                    for k in range(nb):
                                        nc.tensor.transpose(
                                            ps_t[:, k * 128:(k + 1) * 128],
                                            Pn[:, (lj + k) * 128:(lj + k + 1) * 128],
                                            ident[:])
                                    nc.vector.tensor_copy(
                                        pts_v[:, idx0 + lj: idx0 + lj + nb,
                                              h * 2 + half, :],
                                        ps_t[:, 0:nb * 128].rearrange(
                                            "p (k c) -> p k c", c=128))
                                    lj += nb
                        if pending is not None:
                            emit_pv(*pending)
                        pending = (g, ginfo, pts_all)

                    for g in range(NT // 2):
                        if g < 2:
                            with tc.high_priority(offset=330):
                                emit_group(g)
                        else:
                            emit_group(g)
                    emit_pv(*pending)

                if debug_taps:
                    for c in range(4):
                        nc.sync.dma_start(
                            out=taps["encT_tap"][c * 128:(c + 1) * 128, :],
                            in_=encT[c][:].bitcast(F32))
                ps_shared.__exit__(None, None, None)

                # ---------------- stage C: output projection ----------------------
                with (
                    tc.tile_pool(name="outp", bufs=2) as outp,
                    tc.tile_pool(name="psO", bufs=2, space="PSUM") as psO,
                ):
                    for tt in range(NT):
                        ops = psO.tile([128, T], F32, tag="ops", name=f"ops{tt}")
                        for cc in range(4):
                            for nb in range(4):
                                nc.tensor.matmul(
                                    ops[:, nb * 512:(nb + 1) * 512],
                                    encT[cc][:, tt * 128:(tt + 1) * 128],
                                    wout_t[cc][:, nb * 512:(nb + 1) * 512],
                                    start=(cc == 0), stop=(cc == 3),
                                )
                        ob = outp.tile([128, T], F16, tag="ob", name=f"ob{tt}")
                        nc.scalar.activation(ob[:], ops[:], ACTF.Copy)
                        for nb in range(4):
                            nc.sync.dma_start(
                                out=yp[tt * 128:(tt + 1) * 128,
                                       nb * 512:(nb + 1) * 512],
                                in_=ob[:, nb * 512:(nb + 1) * 512])

    nc.compile()
    return nc


def kernel(x, positions, attn_mask, wq, wkv, wout, q_scale, k_scale):
    BF = ml_dtypes.bfloat16
    x = np.ascontiguousarray(x, np.float32)
    positions = np.asarray(positions)
    wq = np.ascontiguousarray(wq, np.float32)
    wkv = np.ascontiguousarray(wkv, np.float32)
    wout = np.ascontiguousarray(wout, np.float32)
    q_scale = np.asarray(q_scale, np.float32)
    k_scale = np.asarray(k_scale, np.float32)

    valid, Wb, js = _geometry(positions, attn_mask)
    shared = not (q_scale.any() or k_scale.any())

    key = (Wb, js, shared, DEBUG_TAPS)
    if key not in _prog_cache:
        _prog_cache[key] = _build(Wb, js, shared, DEBUG_TAPS)
    nc = _prog_cache[key]

    # host-side bias bands: 0 where valid, MASK_NEG elsewhere (incl. padding)
    bias = np.full((B, NT, 128, Wb), MASK_NEG, np.float32)
    for it in range(NT):
        j0 = js[it]
        w = min(Wb, T - j0)
        vslab = valid[:, it * 128:(it + 1) * 128, j0:j0 + w]
        bias[:, it, :, :w][vslab] = 0.0
    bias = bias.astype(ml_dtypes.bfloat16)

    ident = np.eye(128, dtype=BF)
    ones1 = np.ones((1, 128), np.float32)
    onesc = np.ones((128, 1), np.float32)

    in_maps = []
    for core in range(8):
        b, kh = divmod(core, NUM_KV_HEADS)
        m = {
            "xT": np.ascontiguousarray(x[b].T.astype(BF)),
            "wq": np.ascontiguousarray(wq[:, kh * 512:(kh + 1) * 512].astype(BF)),
            "wk": np.ascontiguousarray(wkv[:, kh * 256:(kh + 1) * 256].astype(BF)),
            "wv": np.ascontiguousarray(
                wkv[:, 1024 + kh * 256: 1024 + (kh + 1) * 256].astype(BF)),
            "wout": np.ascontiguousarray(wout[kh * 512:(kh + 1) * 512, :].astype(BF)),
            "ident": ident, "ones1": ones1, "onesc": onesc,
            "bias": bias[b],
        }
        if shared:
            ct, st, _, _ = _rope_tables(positions[b], np.zeros(HEAD_DIM, np.float32))
            m["ct"], m["st"] = ct.astype(BF), st.astype(BF)
        else:
            for nm, tb in zip(("cq1", "sq1", "cq2", "sq2"),
                              _rope_tables(positions[b], q_scale)):
                m[nm] = tb.astype(BF)
            for nm, tb in zip(("ck1", "sk1", "ck2", "sk2"),
                              _rope_tables(positions[b], k_scale)):
                m[nm] = tb.astype(BF)
        in_maps.append(m)

    res = run_bass_kernel_spmd(nc, in_maps, list(range(8)))
    kernel._last_results = res
    out = np.empty((B, T, T), np.float32)
    for b in range(B):
        acc = res.results[b * NUM_KV_HEADS]["yp"].astype(np.float32)
        for kh in range(1, NUM_KV_HEADS):
            acc += res.results[b * NUM_KV_HEADS + kh]["yp"].astype(np.float32)
        out[b] = acc
    return out



# revision 29
# speedup vs baseline: 1.4257x; 1.0428x over previous
"""Trainium2 Bass kernel for nn_Attention_83081847374268 (sparse sliding-window GQA).

Sharding: 8 cores = batch (2, data parallel) x kv-head (4, tensor parallel).
Each core computes, for its (b, kh): q/k/v projections (2 q heads, 1 kv head),
QK-RMSNorm + RoPE, banded sliding-window attention, and a partial output
projection against its 512-row slice of wout.  The host sums the 4 partials
per batch (the TP reduction) and stacks the batches.

Device dataflow (per core):
  stage A: stream xT column-chunks; matmul projections directly in transposed
           layout (qT/kT [head_dim, T]); RMSNorm via ones-matmul variance +
           PE-broadcast rstd; RoPE fused with the rstd multiply on DVE.
           v computed in natural layout [T, 256] and bounced via DRAM.
  stage B: per 128-query tile: S = qT.T @ kT over a host-chosen key window,
           additive mask bias (host-precomputed, handles any attn_mask /
           positions pattern), exp on ACT with fused row-sum, normalize on
           DVE, PE-transpose P, PV matmuls accumulate encoded^T.
  stage C: out partial = encT.T @ wout_slice, DMA per 128-row tile.

All matmuls run as float32r (TRN2 rounds fp32r operands to 12 mantissa bits
on write; full speed at moving-dim >= 256).
"""
import sys

sys.path.insert(0, "/opt/trn_rl_repo")

import numpy as np
import ml_dtypes

import concourse.bacc as bacc
import concourse.mybir as mybir
from concourse.bass_utils import run_bass_kernel_spmd
from concourse.tile import TileContext
from concourse.alu_op_type import AluOpType

F32 = mybir.dt.float32
F32R = mybir.dt.float32r
BF16 = mybir.dt.bfloat16
F16 = mybir.dt.float16
ACTF = mybir.ActivationFunctionType

B, T, WIDTH = 2, 2048, 2048
NUM_HEADS, NUM_KV_HEADS, HEAD_DIM = 8, 4, 256
GROUPS = NUM_HEADS // NUM_KV_HEADS  # 2 q heads per kv head (= per core)
WINDOW = 512
ROPE_BASE = 10000.0
ALPHA = HEAD_DIM ** -0.5
MASK_NEG = -100.0  # exp(S/16 + MASK_NEG) == 0 for |S|<=~16; exact in bf16

NT = T // 128           # 16 query tiles
TCH = 512               # stage-A t-chunk width
NTCH = T // TCH         # 4
NW = WIDTH // 128       # 16 contraction chunks

_prog_cache = {}
DEBUG_TAPS = False


def _round_up(x, m):
    return (x + m - 1) // m * m


def _geometry(positions, attn_mask):
    """Per-query-tile key windows from the actual mask/positions data."""
    pos = np.asarray(positions)
    am = np.asarray(attn_mask)
    pd = pos[:, :, None].astype(np.int64) - pos[:, None, :].astype(np.int64)
    valid = am & (np.abs(pd) < WINDOW)  # [B, T, T] bool
    assert valid.any(axis=2).all(), "a query row with no valid key is unsupported"
    js = []
    wmax = 0
    for it in range(NT):
        cols = valid[:, it * 128:(it + 1) * 128, :].any(axis=(0, 1))
        idx = np.nonzero(cols)[0]
        j_lo, j_hi = int(idx[0]), int(idx[-1]) + 1
        j0 = (j_lo // 128) * 128
        wmax = max(wmax, j_hi - j0)
        js.append(j0)
    Wb = max(256, _round_up(wmax, 128))
    Wb = min(Wb, T)
    js = tuple(max(0, min(j, T - Wb)) for j in js)
    return valid, Wb, js


def _pieces(w):
    """Split w (multiple of 128) into moving-dim pieces of <=512 (PSUM bank)."""
    out = []
    rem = w
    while rem > 512:
        take = 512 if rem - 512 >= 256 or rem == 1024 else rem - 256
        out.append(take)
        rem -= take
    if rem:
        out.append(rem)
    return out


def _classify(valid, js, Wb):
    """Per (query tile, window chunk): 'full' / 'empty' / ('mask', slot).

    A chunk is classified identically for every batch (required: one program
    runs on all cores); mask *contents* are uploaded per core.  Returns the
    plan and the per-batch pattern stack keyed by slot.
    """
    NJ = Wb // 128
    pats = {}
    plan = []
    for it in range(NT):
        base = js[it] // 128
        row = []
        for i in range(NJ):
            jc = base + i
            sl = valid[:, it * 128:(it + 1) * 128, jc * 128:(jc + 1) * 128]
            if sl.all():
                row.append(("full", None))
            elif not sl.any():
                row.append(("empty", None))
            else:
                key = sl.tobytes()
                if key not in pats:
                    pats[key] = (len(pats), sl.copy())
                row.append(("mask", pats[key][0]))
        plan.append(tuple(row))
    patterns = [sl for _, sl in sorted(pats.values(), key=lambda v: v[0])]
    return tuple(plan), patterns


def _runs(row):
    """Contiguous runs of non-empty chunks: list of (c0, c1)."""
    runs = []
    c = 0
    NJ = len(row)
    while c < NJ:
        if row[c][0] == "empty":
            c += 1
            continue
        c0 = c
        while c < NJ and row[c][0] != "empty":
            c += 1
        runs.append((c0, c))
    return runs


def _rope_tables(pos_b, scale):
    """cos/sin tables in [head_dim/2, T] (transposed) layout, gain folded in."""
    d = np.arange(HEAD_DIM // 2, dtype=np.float32)
    timescale = (ROPE_BASE ** (2.0 / HEAD_DIM * d)).astype(np.float32)
    rad = pos_b.astype(np.float32)[None, :] / timescale[:, None]  # [128, T]
    cos, sin = np.cos(rad).astype(np.float32), np.sin(rad).astype(np.float32)
    g1 = (1.0 + scale[:HEAD_DIM // 2]).astype(np.float32)[:, None]
    g2 = (1.0 + scale[HEAD_DIM // 2:]).astype(np.float32)[:, None]
    # o1 = a1*C1 - a2*S2 ; o2 = a2*C2 + a1*S1
    return (cos * g1, sin * g1, cos * g2, sin * g2)  # C1, S1, C2, S2


def _build(Wb, js, plan, n_pat, shared_tables, debug_taps=False):
    nc = bacc.Bacc("TRN2", target_bir_lowering=False, debug=False, num_devices=8)

    def din(name, shape, dt):
        return nc.dram_tensor(name, shape, dt, kind="ExternalInput").ap()

    xT = din("xT", [WIDTH, T], BF16)
    wq = din("wq", [WIDTH, 512], BF16)
    wk = din("wk", [WIDTH, 256], BF16)
    wv = din("wv", [WIDTH, 256], BF16)
    wout = din("wout", [512, T], BF16)
    ident_d = din("ident", [128, 128], BF16)
    ones1_d = din("ones1", [1, 128], F32R)    # K=1 broadcast lhsT
    onesc_d = din("onesc", [128, 1], F32R)    # partition-sum lhsT
    masks_d = din("masks", [max(n_pat, 1), 128, 128], BF16)
    tab_names = ["ct", "st"] if shared_tables else [
        "cq1", "sq1", "cq2", "sq2", "ck1", "sk1", "ck2", "sk2"]
    tabs = {n: din(n, [128, T], BF16) for n in tab_names}
    yp = nc.dram_tensor("yp", [T, T], F16, kind="ExternalOutput").ap()

    taps = {}
    if debug_taps:
        for nm, shape in (("qT_tap", [512, T]), ("kT_tap", [256, T]),
                          ("v_tap", [T, 256]), ("encT_tap", [512, T])):
            taps[nm] = nc.dram_tensor(nm, shape, F32, kind="ExternalOutput").ap()

    NJ = Wb // 128  # window chunks per tile
    # per-group (2 query tiles) union of key chunks, and whether each half
    # contributes a LIVE (non-empty) block there
    groups = []
    for g in range(NT // 2):
        def live(it):
            base = js[it] // 128
            return {base + i for i in range(NJ) if plan[it][i][0] != "empty"}
        w0, w1 = live(2 * g), live(2 * g + 1)
        groups.append([(jc, jc in w0, jc in w1) for jc in sorted(w0 | w1)])

    with TileContext(nc) as tc:
        with (
            tc.tile_pool(name="persist", bufs=1) as pp,
            tc.tile_pool(name="qk_store", bufs=1) as qkp,
        ):
            ident = pp.tile([128, 128], BF16)
            nc.sync.dma_start(out=ident[:], in_=ident_d[:])
            ones1 = pp.tile([1, 128], F32R)
            nc.sync.dma_start(out=ones1[:], in_=ones1_d[:])
            onesc = pp.tile([128, 1], F32R)
            nc.sync.dma_start(out=onesc[:], in_=onesc_d[:])
            epsb = pp.tile([1, 1], F32)
            nc.any.memset(epsb[:], 1e-6)
            epsbq = pp.tile([1, 1], F32)
            nc.any.memset(epsbq[:], HEAD_DIM * 1e-6)
            ones_f = pp.tile([1, 1], F32)
            nc.any.memset(ones_f[:], 1.0)
            rstdq_c = [pp.tile([128, NT], F32, tag=f"rstdq{hh}", name=f"rstdq{hh}")
                       for hh in range(2)]
            qT = [qkp.tile([128, T], BF16, tag=f"qT{c}", name=f"qT{c}") for c in range(4)]
            kT = [qkp.tile([128, T], BF16, tag=f"kT{c}", name=f"kT{c}") for c in range(2)]
            # v resident in SBUF, natural [keys, head_dim] layout per 128-chunk
            vt = qkp.tile([128, NT * 256], BF16, tag="vt", name="vt")

            # ---------------- stage A: projections + RMSNorm + RoPE ----------
            ps_shared = tc.tile_pool(name="ps_shared", bufs=2, space="PSUM")
            psA = psA1 = psS = psT = psE = ps_shared.__enter__()
            with (
                tc.tile_pool(name="wpool", bufs=1) as wp,
                tc.tile_pool(name="xpool", bufs=2) as xp,
                tc.tile_pool(name="tabpool", bufs=1) as tp,
                tc.tile_pool(name="sa", bufs=1) as sa,
            ):
                wq_t = wp.tile([128, NW * 512], BF16)
                wk_t = wp.tile([128, NW * 256], BF16)
                wv_t = wp.tile([128, NW * 256], BF16)
                wq_r = wq.rearrange("(c p) m -> p c m", p=128)
                wk_r = wk.rearrange("(c p) m -> p c m", p=128)
                wv_r = wv.rearrange("(c p) m -> p c m", p=128)
                wq_v = wq_t[:].rearrange("p (c m) -> p c m", m=512)
                wk_v = wk_t[:].rearrange("p (c m) -> p c m", m=256)
                wv_v = wv_t[:].rearrange("p (c m) -> p c m", m=256)
                xT_r = xT.rearrange("(c p) t -> p c t", p=128)

                def load_xts(tci):
                    t0 = tci * TCH
                    xts = xp.tile([128, NW * TCH], BF16, tag="xts", name=f"xts{tci}")
                    xv = xts[:].rearrange("p (c t) -> p c t", t=TCH)
                    for q4 in range(4):
                        nc.sync.dma_start(
                            out=xv[:, q4 * 4:(q4 + 1) * 4],
                            in_=xT_r[:, q4 * 4:(q4 + 1) * 4, t0:t0 + TCH],
                        )
                    return xts

                xts_pre = xp.tile([128, NW * TCH], BF16, tag="xts", name="xts0")
                xv0 = xts_pre[:].rearrange("p (c t) -> p c t", t=TCH)
                # DMA issue order follows first-use order on the PE:
                # k unit (wk+x0), v unit (wv), rope tables, q units (wq).
                for q4 in range(4):
                    nc.sync.dma_start(out=wk_v[:, q4 * 4:(q4 + 1) * 4],
                                      in_=wk_r[:, q4 * 4:(q4 + 1) * 4])
                    nc.sync.dma_start(out=xv0[:, q4 * 4:(q4 + 1) * 4],
                                      in_=xT_r[:, q4 * 4:(q4 + 1) * 4, 0:TCH])
                for q4 in range(4):
                    nc.sync.dma_start(out=wv_v[:, q4 * 4:(q4 + 1) * 4],
                                      in_=wv_r[:, q4 * 4:(q4 + 1) * 4])

                if shared_tables:
                    q_tabs = k_tabs = ("ct", "st", "ct", "st")
                else:
                    q_tabs = ("cq1", "sq1", "cq2", "sq2")
                    k_tabs = ("ck1", "sk1", "ck2", "sk2")
                units = [
                    (wk_t, 256, 0, k_tabs, kT, 0, None),
                    (wq_t, 512, 0, q_tabs, qT, 0, 0),
                    (wq_t, 512, 256, q_tabs, qT, 2, 1),
                ]
                def emit_v(tci, xts):
                    t0 = tci * TCH
                    vT_sb = sa.tile([128, 2 * TCH], BF16, tag="vTsb")
                    for cc in range(2):
                        psv = psA.tile([128, TCH], F32, tag="t_eps")
                        for wc in range(NW):
                            nc.tensor.matmul(
                                psv[:],
                                wv_t[:, wc * 256 + cc * 128: wc * 256 + (cc + 1) * 128],
                                xts[:, wc * TCH:(wc + 1) * TCH],
                                start=(wc == 0), stop=(wc == NW - 1),
                            )
                        nc.scalar.activation(vT_sb[:, cc * TCH:(cc + 1) * TCH],
                                             psv[:], ACTF.Copy)
                    for s in range(TCH // 128):
                        jc = tci * (TCH // 128) + s
                        for cc in range(2):
                            psvt = psA1.tile([128, 128], BF16, tag="t_aux")
                            nc.tensor.transpose(
                                psvt[:],
                                vT_sb[:, cc * TCH + s * 128: cc * TCH + (s + 1) * 128],
                                ident[:])
                            nc.vector.tensor_copy(
                                vt[:, jc * 256 + cc * 128: jc * 256 + (cc + 1) * 128],
                                psvt[:])

                for tci in range(NTCH):
                    t0 = tci * TCH
                    xts = xts_pre if tci == 0 else load_xts(tci)
                    # q/k projections in transposed layout + norm + rope
                    tabt = {}
                    for name in dict.fromkeys(q_tabs + k_tabs):
                        tt = tp.tile([128, TCH], BF16, tag=name, name=f"tab_{name}")
                        nc.sync.dma_start(out=tt[:], in_=tabs[name][:, t0:t0 + TCH])
                        tabt[name] = tt
                    if tci == 0:
                        for q4 in range(4):
                            nc.sync.dma_start(out=wq_v[:, q4 * 4:(q4 + 1) * 4],
                                              in_=wq_r[:, q4 * 4:(q4 + 1) * 4])
                    for unit_i, (w_t, wcols, cbase, tkeys, dest, dbase, qhead) in \
                            enumerate(units):
                        if unit_i == 1:
                            emit_v(tci, xts)  # v right after k: wv arrives before wq
                        ps1 = psA.tile([128, TCH], F32, tag="t_s0")
                        ps2 = psA.tile([128, TCH], F32, tag="t_s1")
                        for ps, cc in ((ps1, 0), (ps2, 1)):
                            coff = cbase + cc * 128
                            for wc in range(NW):
                                nc.tensor.matmul(
                                    ps[:],
                                    w_t[:, wc * wcols + coff: wc * wcols + coff + 128],
                                    xts[:, wc * TCH:(wc + 1) * TCH],
                                    start=(wc == 0), stop=(wc == NW - 1),
                                )
                        sq1 = sa.tile([128, TCH], F32R, tag="sq1")
                        sq2 = sa.tile([128, TCH], F32R, tag="sq2")
                        nc.scalar.activation(sq1[:], ps1[:], ACTF.Square)
                        nc.scalar.activation(sq2[:], ps2[:], ACTF.Square)
                        psvar = psA1.tile([1, TCH], F32, tag="t_aux")
                        nc.tensor.matmul(psvar[:], onesc[:], sq1[:], start=True, stop=False)
                        nc.tensor.matmul(psvar[:], onesc[:], sq2[:], start=False, stop=True)
                        C1, S1, C2, S2 = (tabt[k] for k in tkeys)
                        m1 = sa.tile([128, TCH], F32, tag="m1")
                        m2 = sa.tile([128, TCH], F32, tag="m2")
                        m3 = sa.tile([128, TCH], F32, tag="m1", name="m3t")
                        m4 = sa.tile([128, TCH], F32, tag="m2", name="m4t")
                        if qhead is None:
                            # k: apply rstd via PE broadcast, fused into rope
                            stdv = sa.tile([1, TCH], F32R, tag="stdv")
                            nc.scalar.activation(stdv[:], psvar[:], ACTF.Sqrt,
                                                 scale=1.0 / HEAD_DIM, bias=epsb[:])
                            psb = psA1.tile([128, TCH], F32, tag="t_aux")
                            nc.tensor.matmul(psb[:], ones1[:], stdv[:],
                                             start=True, stop=True)
                            rb = sa.tile([128, TCH], F32, tag="rb")
                            nc.vector.reciprocal_approx_fast(out=rb[:], in_=psb[:])
                            a1 = sa.tile([128, TCH], F32, tag="a1")
                            a2 = sa.tile([128, TCH], F32, tag="a2")
                            nc.vector.tensor_tensor(a1[:], ps1[:], rb[:], AluOpType.mult)
                            nc.vector.tensor_tensor(a2[:], ps2[:], rb[:], AluOpType.mult)
                        else:
                            # q: defer 1/std to the stage-B logits scale;
                            # transpose 16*std per 128-tile via K=1 matmuls
                            stdvf = sa.tile([1, TCH], F32, tag="stdvf")
                            nc.scalar.activation(stdvf[:], psvar[:], ACTF.Sqrt,
                                                 bias=epsbq[:])
                            sq_ps = psA1.tile([128, TCH // 128], F32, tag="t_aux")
                            for s in range(TCH // 128):
                                nc.tensor.matmul(
                                    sq_ps[:, s:s + 1],
                                    stdvf[:, s * 128:(s + 1) * 128],
                                    ones_f[:], start=True, stop=True)
                            stdq = sa.tile([128, TCH // 128], F32, tag="stdq")
                            nc.scalar.activation(stdq[:], sq_ps[:], ACTF.Copy)
                            nc.vector.reciprocal_approx_fast(
                                out=rstdq_c[qhead][:, tci * (TCH // 128):
                                                   (tci + 1) * (TCH // 128)],
                                in_=stdq[:])
                            a1, a2 = ps1, ps2
                        nc.vector.tensor_tensor(m1[:], a1[:], C1[:], AluOpType.mult)
                        nc.vector.tensor_tensor(m2[:], a2[:], S2[:], AluOpType.mult)
                        nc.vector.tensor_tensor(
                            dest[dbase][:, t0:t0 + TCH], m1[:], m2[:], AluOpType.subtract)
                        nc.vector.tensor_tensor(m3[:], a2[:], C2[:], AluOpType.mult)
                        nc.vector.tensor_tensor(m4[:], a1[:], S1[:], AluOpType.mult)
                        nc.vector.tensor_tensor(
                            dest[dbase + 1][:, t0:t0 + TCH], m3[:], m4[:], AluOpType.add)
            if debug_taps:
                for c in range(4):
                    nc.sync.dma_start(out=taps["qT_tap"][c * 128:(c + 1) * 128, :],
                                      in_=qT[c][:].bitcast(F32))
                for c in range(2):
                    nc.sync.dma_start(out=taps["kT_tap"][c * 128:(c + 1) * 128, :],
                                      in_=kT[c][:].bitcast(F32))

            # ---------------- stage B: banded attention ----------------------
            with (
                tc.tile_pool(name="encp", bufs=1) as encp,
                tc.tile_pool(name="woutp", bufs=1) as woutp,
            ):
                encT = [encp.tile([128, T], BF16, tag=f"encT{c}", name=f"encT{c}") for c in range(4)]
                wout_t = [woutp.tile([128, T], BF16, tag=f"wo{c}", name=f"wo{c}") for c in range(4)]
                wout_r = wout.rearrange("(c p) t -> c p t", p=128)
                for c in range(4):
                    nc.sync.dma_start(out=wout_t[c][:], in_=wout_r[c])

                maskt = encp.tile([128, max(n_pat, 1) * 128], BF16,
                                  tag="maskt", name="maskt")
                for p in range(n_pat):
                    nc.sync.dma_start(out=maskt[:, p * 128:(p + 1) * 128],
                                      in_=masks_d[p])

                with (
                    tc.tile_pool(name="sb", bufs=2) as sbp,
                    tc.tile_pool(name="ptp", bufs=2) as ptp,
                    tc.tile_pool(name="outp", bufs=3) as outp,
                ):
                    def emit_pv(g, ginfo, pts_all):
                        for i, (jc, inA, inB) in enumerate(ginfo):
                            for h in range(2):
                                if not inA:
                                    nc.gpsimd.memset(
                                        pts_all[:, i * 512 + h * 256:
                                                i * 512 + h * 256 + 128], 0.0)
                                if not inB:
                                    nc.gpsimd.memset(
                                        pts_all[:, i * 512 + h * 256 + 128:
                                                i * 512 + h * 256 + 256], 0.0)
                        for cc in range(2):
                            eps = psE.tile([128, 512], F32, tag="t_eps", name=f"eps{g}_{cc}")
                            for i, (jc, _, _) in enumerate(ginfo):
                                nc.tensor.matmul(
                                    eps[:],
                                    vt[:, jc * 256 + cc * 128: jc * 256 + (cc + 1) * 128],
                                    pts_all[:, i * 512:(i + 1) * 512],
                                    start=(i == 0), stop=(i == len(ginfo) - 1),
                                )
                            for h in range(2):
                                nc.scalar.activation(
                                    encT[2 * h + cc][:, g * 256:(g + 1) * 256],
                                    eps[:, h * 256:(h + 1) * 256], ACTF.Copy)

                    def emit_out(g):
                        # output projection for this group's two query tiles,
                        # interleaved into stage B (PSUM tag shared with eps)
                        for half in range(2):
                            tt = 2 * g + half
                            for nb in range(4):
                                ops = psE.tile([128, 512], F32, tag="t_eps",
                                               name=f"ops{tt}_{nb}")
                                for cc in range(4):
                                    nc.tensor.matmul(
                                        ops[:],
                                        encT[cc][:, tt * 128:(tt + 1) * 128],
                                        wout_t[cc][:, nb * 512:(nb + 1) * 512],
                                        start=(cc == 0), stop=(cc == 3),
                                    )
                                ob = outp.tile([128, 512], F16, tag="ob",
                                               name=f"ob{tt}_{nb}")
                                if nb % 2 == 0:
                                    nc.scalar.activation(ob[:], ops[:], ACTF.Copy)
                                else:
                                    nc.vector.tensor_copy(ob[:], ops[:])
                                nc.sync.dma_start(
                                    out=yp[tt * 128:(tt + 1) * 128,
                                           nb * 512:(nb + 1) * 512],
                                    in_=ob[:])

                    pdict = {}
                    pending = None

                    def emit_group(g):
                        nonlocal pending
                        ginfo = groups[g]
                        nj = len(ginfo)
                        pts_all = ptp.tile([128, nj * 512], BF16, tag="pts",
                                           name=f"pts{g}")
                        den2 = sbp.tile([128, 2 * 2], F32, tag="den2", name=f"den2_{g}")
                        for half in range(2):
                            it = 2 * g + half
                            jst = js[it]
                            base = jst // 128
                            row = plan[it]
                            runs = _runs(row)
                            for h in range(2):
                                # S pieces over non-empty runs; mask+exp straight
                                # from PSUM (deferred 1/std as per-row exp scale)
                                plist = []
                                for c0, c1 in runs:
                                    col = c0 * 128
                                    for pw in _pieces((c1 - c0) * 128):
                                        ps = psS.tile(
                                            [128, pw], F32,
                                            tag=f"t_s{len(plist) % 2}",
                                            name=f"S{it}_{h}_{len(plist)}")
                                        for cc in range(2):
                                            nc.tensor.matmul(
                                                ps[:],
                                                qT[2 * h + cc][:, it * 128:(it + 1) * 128],
                                                kT[cc][:, jst + col: jst + col + pw],
                                                start=(cc == 0), stop=(cc == 1),
                                            )
                                        plist.append((ps, col, pw))
                                        col += pw
                                for ps, col, pw in plist:
                                    for i in range(col // 128, (col + pw) // 128):
                                        cls, slot = row[i]
                                        if cls == "mask":
                                            off = i * 128 - col
                                            nc.vector.tensor_tensor(
                                                ps[:, off:off + 128],
                                                ps[:, off:off + 128],
                                                maskt[:, slot * 128:(slot + 1) * 128],
                                                AluOpType.add)
                                P_t = sbp.tile([128, Wb], BF16, tag=f"P{h}",
                                               name=f"P{it}_{h}")
                                dslot = den2[:, half * 2 + h: half * 2 + h + 1]
                                dparts = sbp.tile([128, 4], F32, tag=f"dp{h}",
                                                  name=f"dp{it}_{h}")
                                for pi, (ps, col, pw) in enumerate(plist):
                                    nc.scalar.activation(
                                        P_t[:, col:col + pw], ps[:], ACTF.Exp,
                                        scale=rstdq_c[h][:, it:it + 1],
                                        accum_out=(dslot if len(plist) == 1
                                                   else dparts[:, pi:pi + 1]))
                                if len(plist) > 1:
                                    nc.gpsimd.tensor_tensor(
                                        dslot, dparts[:, 0:1], dparts[:, 1:2],
                                        AluOpType.add)
                                    for pi in range(2, len(plist)):
                                        nc.gpsimd.tensor_tensor(
                                            dslot, dslot, dparts[:, pi:pi + 1],
                                            AluOpType.add)
                                pdict[(half, h)] = (P_t, runs)
                            rden = sbp.tile([128, 2], F32, tag="rden", name=f"rden{it}")
                            nc.vector.reciprocal_approx_fast(
                                out=rden[:], in_=den2[:, half * 2: half * 2 + 2])
                            for h in range(2):
                                P_t, truns = pdict[(half, h)]
                                Pn = sbp.tile([128, Wb], BF16, tag="Pn", name=f"Pn{it}_{h}")
                                for c0, c1 in truns:
                                    nc.vector.tensor_scalar_mul(
                                        Pn[:, c0 * 128:c1 * 128],
                                        P_t[:, c0 * 128:c1 * 128], rden[:, h:h + 1])
                                pts_v = pts_all[:].rearrange(
                                    "p (i f c) -> p i f c", f=4, c=128)
                                for c0, c1 in truns:
                                    idx0 = next(i for i, (c, _, _) in enumerate(ginfo)
                                                if c == base + c0)
                                    lj = c0
                                    while lj < c1:
                                        nb = min(3, c1 - lj)
                                        ps_t = psT.tile([128, 3 * 128], BF16, tag="t_aux",
                                                        name=f"ptps{it}_{h}_{lj}")
                                        for k in range(nb):
                                            nc.tensor.transpose(
                                                ps_t[:, k * 128:(k + 1) * 128],
                                                Pn[:, (lj + k) * 128:(lj + k + 1) * 128],
                                                ident[:])
                                        nc.vector.tensor_copy(
                                            pts_v[:, idx0 + lj - c0: idx0 + lj - c0 + nb,
                                                  h * 2 + half, :],
                                            ps_t[:, 0:nb * 128].rearrange(
                                                "p (k c) -> p k c", c=128))
                                        lj += nb
                        if pending is not None:
                            pg = pending
                            emit_pv(*pg)
                            emit_out(pg[0])
                        pending = (g, ginfo, pts_all)

                    for g in range(NT // 2):
                        if g < 2:
                            with tc.high_priority(offset=330):
                                emit_group(g)
                        else:
                            emit_group(g)
                    emit_pv(*pending)
                    emit_out(pending[0])

                if debug_taps:
                    for c in range(4):
                        nc.sync.dma_start(
                            out=taps["encT_tap"][c * 128:(c + 1) * 128, :],
                            in_=encT[c][:].bitcast(F32))
                ps_shared.__exit__(None, None, None)

    nc.compile()
    return nc


def kernel(x, positions, attn_mask, wq, wkv, wout, q_scale, k_scale):
    BF = ml_dtypes.bfloat16
    x = np.ascontiguousarray(x, np.float32)
    positions = np.asarray(positions)
    wq = np.ascontiguousarray(wq, np.float32)
    wkv = np.ascontiguousarray(wkv, np.float32)
    wout = np.ascontiguousarray(wout, np.float32)
    q_scale = np.asarray(q_scale, np.float32)
    k_scale = np.asarray(k_scale, np.float32)

    valid, Wb, js = _geometry(positions, attn_mask)
    shared = not (q_scale.any() or k_scale.any())
    plan, patterns = _classify(valid, js, Wb)
    n_pat = len(patterns)

    key = (Wb, js, plan, n_pat, shared, DEBUG_TAPS)
    if key not in _prog_cache:
        _prog_cache[key] = _build(Wb, js, plan, n_pat, shared, DEBUG_TAPS)
    nc = _prog_cache[key]

    # per-batch additive mask tiles: 0 where valid, -3e4 elsewhere
    masks = np.zeros((B, max(n_pat, 1), 128, 128), np.float32)
    for p, pat in enumerate(patterns):
        masks[:, p] = np.where(pat, 0.0, -3.0e4)
    masks = masks.astype(ml_dtypes.bfloat16)

    ident = np.eye(128, dtype=BF)
    ones1 = np.ones((1, 128), np.float32)
    onesc = np.ones((128, 1), np.float32)

    in_maps = []
    for core in range(8):
        b, kh = divmod(core, NUM_KV_HEADS)
        m = {
            "xT": np.ascontiguousarray(x[b].T.astype(BF)),
            "wq": np.ascontiguousarray(wq[:, kh * 512:(kh + 1) * 512].astype(BF)),
            "wk": np.ascontiguousarray(wkv[:, kh * 256:(kh + 1) * 256].astype(BF)),
            "wv": np.ascontiguousarray(
                wkv[:, 1024 + kh * 256: 1024 + (kh + 1) * 256].astype(BF)),
            "wout": np.ascontiguousarray(wout[kh * 512:(kh + 1) * 512, :].astype(BF)),
            "ident": ident, "ones1": ones1, "onesc": onesc,
            "masks": masks[b],
        }
        if shared:
            ct, st, _, _ = _rope_tables(positions[b], np.zeros(HEAD_DIM, np.float32))
            m["ct"], m["st"] = ct.astype(BF), st.astype(BF)
        else:
            for nm, tb in zip(("cq1", "sq1", "cq2", "sq2"),
                              _rope_tables(positions[b], q_scale)):
                m[nm] = tb.astype(BF)
            for nm, tb in zip(("ck1", "sk1", "ck2", "sk2"),
                              _rope_tables(positions[b], k_scale)):
                m[nm] = tb.astype(BF)
        in_maps.append(m)

    res = run_bass_kernel_spmd(nc, in_maps, list(range(8)))
    kernel._last_results = res
    out = np.empty((B, T, T), np.float32)
    for b in range(B):
        acc = res.results[b * NUM_KV_HEADS]["yp"].astype(np.float32)
        for kh in range(1, NUM_KV_HEADS):
            acc += res.results[b * NUM_KV_HEADS + kh]["yp"].astype(np.float32)
        out[b] = acc
    return out

